# revision 17
# baseline (speedup 1.0000x reference)
"""Trainium2 Bass kernel for the NCA (neural cellular automaton) problem.

Per step: 3x3 conv (16->16 ch, with the reference's transposed-spatial
orientation) folded into the first MLP layer, then 256->256 x3 + 256->16,
residual update. Output: full step history.

Sharding: batch (8) across 8 NeuronCores, no cross-core communication.

Layout per core (one [128,128,16] image):
  - X3 resident SBUF tile [96, 130, 130] fp32 (exact state): three W-shifted
    copies of the zero-padded state, channel-blocks on 32-partition
    boundaries (16 used + 16 zero pad each; engine partition bases must be
    32-aligned). X3r is a bf16 shadow of X3 (cast by DVE copies) that the
    tensor engine reads. The 3x3 conv + first layer becomes 3 accumulating
    matmuls (dh = 0..2) with K=96 contraction and strided moving APs; the
    column taps (dw) live in the partition blocks.
  - Matmuls use bf16 operands (1 cycle/row, overlapped LDWEIGHTS) with fp32
    accumulation in PSUM; the residual state stays exact fp32. Measured
    end-to-end error ~7e-4 absmax vs the fp32 reference.
  - Last layer's stationary is replicated 3x (M=96) so the residual update
    of all three X3 blocks is three same-partition DVE adds from PSUM,
    followed by one DVE cast refreshing all three X3r blocks at once.
  - The chunk stream (32 chunks/step x steps) is software-pipelined 5 deep
    (one slot per MLP stage) so relu latency hides under other chunks'
    matmuls and the tensor engine stays >99% busy.
  - History is written to DRAM in [C, H, W] layout; the host transposes.
"""

import numpy as np

import concourse.bass as bass
import concourse.mybir as mybir
from concourse import bacc
import concourse.tile as tile
from concourse.bass_utils import run_bass_kernel_spmd

C = 16
HID = 256
NL = 3          # interim layers
B = 8
H = 128
W = 128
Hp, Wp = H + 2, W + 2
ROWS = 4        # output rows per chunk
NCH = H // ROWS
NCORES = 8

f32 = mybir.dt.float32
f32r = mybir.dt.float32r
bf16 = mybir.dt.bfloat16
MM_DT = bf16   # matmul operand dtype: bf16 (fast LDWEIGHTS) or f32r
RELU = mybir.ActivationFunctionType.Relu


def build_program(steps: int) -> bass.Bass:
    nc = bacc.Bacc(target_bir_lowering=False)

    x_d = nc.dram_tensor("x", [C, H, W], f32, kind="ExternalInput")
    w1_d = nc.dram_tensor("w1", [96, 3, 2, 128], MM_DT, kind="ExternalInput")
    wi_d = nc.dram_tensor("wi", [128, NL, 2, 2, 128], MM_DT, kind="ExternalInput")
    wl_d = nc.dram_tensor("wl", [128, 2, 96], MM_DT, kind="ExternalInput")
    bb_d = nc.dram_tensor("bias", [128, NL + 1, 2], f32, kind="ExternalInput")
    hist_d = nc.dram_tensor("hist", [steps, C, H, W], f32, kind="ExternalOutput")

    with tile.TileContext(nc) as tc:
        with (
            tc.tile_pool(name="persist", bufs=1) as persist,
            tc.tile_pool(name="acts", bufs=20) as acts,
            tc.tile_pool(name="psumb", bufs=7, space="PSUM") as psumb,
            tc.tile_pool(name="psum5", bufs=1, space="PSUM") as psum5p,
        ):
            X3 = persist.tile([96, Hp, Wp], f32)
            X3r = persist.tile([96, Hp, Wp], MM_DT)
            w1s = persist.tile([96, 3, 2, 128], MM_DT)
            wis = persist.tile([128, NL, 2, 2, 128], MM_DT)
            wls = persist.tile([128, 2, 96], MM_DT)
            bbs = persist.tile([128, NL + 1, 2], f32)

            nc.gpsimd.memset(X3[:, :, :], 0.0)
            nc.sync.dma_start(out=w1s[:, :, :, :], in_=w1_d[:, :, :, :])
            nc.sync.dma_start(out=wis[:, :, :, :, :], in_=wi_d[:, :, :, :, :])
            nc.sync.dma_start(out=wls[:, :, :], in_=wl_d[:, :, :])
            nc.sync.dma_start(out=bbs[:, :, :], in_=bb_d[:, :, :])
            # initial state into the three shifted blocks
            for dw in range(3):
                nc.sync.dma_start(
                    out=X3[32 * dw:32 * dw + 16, 1:129, 2 - dw:130 - dw],
                    in_=x_d[:, :, :],
                )
            # f32r shadow for the tensor engine (rounding copy)
            nc.vector.tensor_copy(X3r[:, :, :], X3[:, :, :])

            def emit_update(k, ps5):
                """Residual update of chunk k (all three X3 blocks) + X3r refresh."""
                r0 = k * ROWS
                # block dw: X3[16dw+c, r+1, ww + (2-dw)] += ds[c, r, ww]
                for dw in range(3):
                    nc.vector.tensor_add(
                        X3[32 * dw:32 * dw + 16, r0 + 1:r0 + ROWS + 1, 2 - dw:130 - dw],
                        X3[32 * dw:32 * dw + 16, r0 + 1:r0 + ROWS + 1, 2 - dw:130 - dw],
                        ps5[32 * dw:32 * dw + 16, :, :],
                    )
                nc.vector.tensor_copy(X3r[:, r0 + 1:r0 + ROWS + 1, :],
                                      X3[:, r0 + 1:r0 + ROWS + 1, :])

            def emit_hist(k, s):
                r0 = k * ROWS
                nc.sync.dma_start(
                    out=hist_d[s, :, r0:r0 + ROWS, :],
                    in_=X3[32:48, r0 + 1:r0 + ROWS + 1, 1:129],
                )

            # 5-deep software pipeline over the global chunk stream: at each
            # slot the PE gets independent matmuls from 5 different chunks, so
            # relu latency of one chunk hides under other chunks' matmuls.
            total = steps * NCH
            st = {}

            def stage0(g):
                s, k = divmod(g, NCH)
                r0 = k * ROWS
                ps1 = [psumb.tile([128, ROWS * W], f32, tag="big", name=f"ps1_{g}_{mc}") for mc in range(2)]
                for mc in range(2):
                    for dh in range(3):
                        nc.tensor.matmul(
                            ps1[mc][:, :],
                            w1s[:, dh, mc, :],
                            X3r[:, r0 + dh:r0 + dh + ROWS, 1:129],
                            start=(dh == 0),
                            stop=(dh == 2),
                        )
                ds = []
                for mc in range(2):
                    d = acts.tile([128, ROWS * W], MM_DT, tag="act", name=f"ds_{g}_{mc}")
                    nc.scalar.activation(d[:, :], ps1[mc][:, :], RELU,
                                         bias=bbs[:, 0, mc:mc + 1])
                    ds.append(d)
                st[g] = ds

            def stage_mid(g, l):
                ds_prev = st[g]
                ps = [psumb.tile([128, ROWS * W], f32, tag="big", name=f"ps_{g}_{l}_{mc}") for mc in range(2)]
                for mc in range(2):
                    for kc in range(2):
                        nc.tensor.matmul(
                            ps[mc][:, :],
                            wis[:, l, kc, mc, :],
                            ds_prev[kc][:, :],
                            start=(kc == 0),
                            stop=(kc == 1),
                        )
                ds_new = []
                for mc in range(2):
                    d = acts.tile([128, ROWS * W], MM_DT, tag="act", name=f"dsn_{g}_{l}_{mc}")
                    if l < 2 and mc == 0:
                        nc.vector.tensor_scalar(
                            d[:, :], ps[mc][:, :], bbs[:, l + 1, mc:mc + 1],
                            0.0, mybir.AluOpType.add, mybir.AluOpType.max)
                    else:
                        nc.scalar.activation(d[:, :], ps[mc][:, :], RELU,
                                             bias=bbs[:, l + 1, mc:mc + 1])
                    ds_new.append(d)
                st[g] = ds_new

            def stage4(g):
                s, k = divmod(g, NCH)
                ds_prev = st.pop(g)
                ps5 = psum5p.tile([96, ROWS, W], f32, tag="small", name=f"ps5_{g}")
                for kc in range(2):
                    nc.tensor.matmul(
                        ps5[:, :, :],
                        wls[:, kc, :],
                        ds_prev[kc][:, :],
                        start=(kc == 0),
                        stop=(kc == 1),
                    )
                emit_update(k, ps5)
                emit_hist(k, s)

            for slot in range(total + 4):
                if slot < total:
                    stage0(slot)
                if 1 <= slot < total + 1:
                    stage_mid(slot - 1, 0)
                if 2 <= slot < total + 2:
                    stage_mid(slot - 2, 1)
                if 3 <= slot < total + 3:
                    stage_mid(slot - 3, 2)
                if slot >= 4:
                    stage4(slot - 4)

    if not nc.is_finalized():
        nc.finalize()
    return nc


def prepare_host_inputs(x, conv_w, w_first, b_first, w_interim, b_interim, w_last):
    """Pack weights into the kernel's SBUF layouts (numpy, fp32)."""
    conv_w = np.asarray(conv_w, np.float32)
    w_first = np.asarray(w_first, np.float32)
    b_first = np.asarray(b_first, np.float32)
    w_interim = np.asarray(w_interim, np.float32)
    b_interim = np.asarray(b_interim, np.float32)
    w_last = np.asarray(w_last, np.float32)

    # W1c[dh, 16*dw + c, m] = sum_o conv_w[o, c, kh=dw, kw=dh] * w_first[m, o]
    t = np.einsum("ocuv,mo->vucm", conv_w, w_first)      # [dh, dw, c, m]
    W1c = np.zeros((3, 96, HID), np.float32)
    for dw in range(3):
        W1c[:, 32 * dw:32 * dw + 16, :] = t[:, dw]
    w1_host = np.ascontiguousarray(
        W1c.transpose(1, 0, 2).reshape(96, 3, 2, 128))    # [k, dh, mc, mm]

    wi_host = np.ascontiguousarray(
        w_interim.reshape(NL, 2, 128, 2, 128).transpose(4, 0, 3, 1, 2))
    # wi_host[kk, l, kc, mc, mm] = w_interim[l, mc*128+mm, kc*128+kk]

    wl_t = w_last.reshape(16, 2, 128).transpose(2, 1, 0)   # [k, kc, c]
    wl_host = np.zeros((128, 2, 96), np.float32)
    for t_ in range(3):
        wl_host[:, :, 32 * t_:32 * t_ + 16] = wl_t

    b_all = [b_first] + [b_interim[l] for l in range(NL)]
    bias_host = np.ascontiguousarray(
        np.stack([b.reshape(2, 128).T for b in b_all], axis=1))  # [128, li, mc]

    x = np.asarray(x, np.float32)
    x_cores = [np.ascontiguousarray(x[i].transpose(2, 0, 1)) for i in range(B)]
    wdt = mybir.dt.np(MM_DT)
    w1_host = w1_host.astype(wdt)
    wi_host = wi_host.astype(wdt)
    wl_host = wl_host.astype(wdt)
    return x_cores, w1_host, wi_host, wl_host, bias_host


def run(inputs: dict, trace: bool = False):
    """Build, run on 8 cores, return ((x_final, history), BassKernelResults)."""
    x = np.asarray(inputs["x"], np.float32)
    steps = int(inputs["steps"])
    if steps == 0:
        hist = x[None].copy()
        return (x.copy(), hist), None

    x_cores, w1h, wih, wlh, bh = prepare_host_inputs(
        x, inputs["conv_w"], inputs["w_first"], inputs["b_first"],
        inputs["w_interim"], inputs["b_interim"], inputs["w_last"])

    nc = build_program(steps)
    in_maps = [
        {"x": x_cores[i], "w1": w1h, "wi": wih, "wl": wlh, "bias": bh}
        for i in range(NCORES)
    ]
    res = run_bass_kernel_spmd(nc, in_maps, list(range(NCORES)), trace=trace)

    history = np.empty((steps + 1, B, H, W, C), np.float32)
    history[0] = x
    for i in range(NCORES):
        hist_i = res.results[i]["hist"]  # [steps, C, H, W]
        history[1:, i] = hist_i.transpose(0, 2, 3, 1)
    x_final = history[steps].copy()
    return (x_final, history), res


def kernel(x, conv_w, w_first, b_first, w_interim, b_interim, w_last, steps):
    (x_final, history), _ = run(dict(
        x=x, conv_w=conv_w, w_first=w_first, b_first=b_first,
        w_interim=w_interim, b_interim=b_interim, w_last=w_last, steps=steps))
    return x_final, history


# revision 18
# speedup vs baseline: 1.0042x; 1.0042x over previous
"""Trainium2 Bass kernel for the NCA (neural cellular automaton) problem.

Per step: 3x3 conv (16->16 ch, with the reference's transposed-spatial
orientation) folded into the first MLP layer, then 256->256 x3 + 256->16,
residual update. Output: full step history.

Sharding: batch (8) across 8 NeuronCores, no cross-core communication.

Layout per core (one [128,128,16] image):
  - X3 resident SBUF tile [96, 130, 130] fp32 (exact state): three W-shifted
    copies of the zero-padded state, channel-blocks on 32-partition
    boundaries (16 used + 16 zero pad each; engine partition bases must be
    32-aligned). X3r is a bf16 shadow of X3 (cast by DVE copies) that the
    tensor engine reads. The 3x3 conv + first layer becomes 3 accumulating
    matmuls (dh = 0..2) with K=96 contraction and strided moving APs; the
    column taps (dw) live in the partition blocks.
  - Matmuls use bf16 operands (1 cycle/row, overlapped LDWEIGHTS) with fp32
    accumulation in PSUM; the residual state stays exact fp32. Measured
    end-to-end error ~7e-4 absmax vs the fp32 reference.
  - Last layer's stationary is replicated 3x (M=96) so the residual update
    of all three X3 blocks is three same-partition DVE adds from PSUM,
    followed by one DVE cast refreshing all three X3r blocks at once.
  - The chunk stream (32 chunks/step x steps) is software-pipelined 5 deep
    (one slot per MLP stage) so relu latency hides under other chunks'
    matmuls and the tensor engine stays >99% busy.
  - History is written to DRAM in [C, H, W] layout; the host transposes.
"""

import numpy as np

import concourse.bass as bass
import concourse.mybir as mybir
from concourse import bacc
import concourse.tile as tile
from concourse.bass_utils import run_bass_kernel_spmd

C = 16
HID = 256
NL = 3          # interim layers
B = 8
H = 128
W = 128
Hp, Wp = H + 2, W + 2
ROWS = 4        # output rows per chunk
NCH = H // ROWS
NCORES = 8

f32 = mybir.dt.float32
f32r = mybir.dt.float32r
bf16 = mybir.dt.bfloat16
MM_DT = bf16   # matmul operand dtype: bf16 (fast LDWEIGHTS) or f32r
RELU = mybir.ActivationFunctionType.Relu


def build_program(steps: int) -> bass.Bass:
    nc = bacc.Bacc(target_bir_lowering=False)

    xb_d = nc.dram_tensor("xb", [3, 32, Hp, Wp], f32, kind="ExternalInput")
    xbr_d = nc.dram_tensor("xbr", [3, 32, Hp, Wp], MM_DT, kind="ExternalInput")
    w1_d = nc.dram_tensor("w1", [96, 3, 2, 128], MM_DT, kind="ExternalInput")
    wi_d = nc.dram_tensor("wi", [128, NL, 2, 2, 128], MM_DT, kind="ExternalInput")
    wl_d = nc.dram_tensor("wl", [128, 2, 96], MM_DT, kind="ExternalInput")
    bb_d = nc.dram_tensor("bias", [128, NL + 1, 2], f32, kind="ExternalInput")
    hist_d = nc.dram_tensor("hist", [steps, C, H, W], f32, kind="ExternalOutput")

    with tile.TileContext(nc) as tc:
        with (
            tc.tile_pool(name="persist", bufs=1) as persist,
            tc.tile_pool(name="acts", bufs=20) as acts,
            tc.tile_pool(name="psumb", bufs=6, space="PSUM") as psumb,
            tc.tile_pool(name="psum5", bufs=2, space="PSUM") as psum5p,
        ):
            X3 = persist.tile([96, Hp, Wp], f32)
            X3r = persist.tile([96, Hp, Wp], MM_DT)
            w1s = persist.tile([96, 3, 2, 128], MM_DT)
            wis = persist.tile([128, NL, 2, 2, 128], MM_DT)
            wls = persist.tile([128, 2, 96], MM_DT)
            bbs = persist.tile([128, NL + 1, 2], f32)

            nc.sync.dma_start(out=w1s[:, :, :, :], in_=w1_d[:, :, :, :])
            nc.sync.dma_start(out=wis[:, :, :, :, :], in_=wi_d[:, :, :, :, :])
            nc.sync.dma_start(out=wls[:, :, :], in_=wl_d[:, :, :])
            nc.sync.dma_start(out=bbs[:, :, :], in_=bb_d[:, :, :])
            # initial state: host-precomputed shifted blocks (borders and
            # pad partitions pre-zeroed), fp32 + bf16 copies
            for dw in range(3):
                nc.sync.dma_start(out=X3[32 * dw:32 * dw + 32, :, :],
                                  in_=xb_d[dw, :, :, :])
                nc.sync.dma_start(out=X3r[32 * dw:32 * dw + 32, :, :],
                                  in_=xbr_d[dw, :, :, :])

            def emit_update(k, ps5):
                """Residual update of chunk k (all three X3 blocks) + X3r refresh."""
                r0 = k * ROWS
                # block dw: X3[16dw+c, r+1, ww + (2-dw)] += ds[c, r, ww]
                for dw in range(3):
                    nc.vector.tensor_add(
                        X3[32 * dw:32 * dw + 16, r0 + 1:r0 + ROWS + 1, 2 - dw:130 - dw],
                        X3[32 * dw:32 * dw + 16, r0 + 1:r0 + ROWS + 1, 2 - dw:130 - dw],
                        ps5[32 * dw:32 * dw + 16, :, :],
                    )
                nc.vector.tensor_copy(X3r[:, r0 + 1:r0 + ROWS + 1, :],
                                      X3[:, r0 + 1:r0 + ROWS + 1, :])

            def emit_hist(k, s):
                r0 = k * ROWS
                nc.sync.dma_start(
                    out=hist_d[s, :, r0:r0 + ROWS, :],
                    in_=X3[32:48, r0 + 1:r0 + ROWS + 1, 1:129],
                )

            # 5-deep software pipeline over the global chunk stream: at each
            # slot the PE gets independent matmuls from 5 different chunks, so
            # relu latency of one chunk hides under other chunks' matmuls.
            total = steps * NCH
            st = {}

            def stage0(g):
                s, k = divmod(g, NCH)
                r0 = k * ROWS
                ps1 = [psumb.tile([128, ROWS * W], f32, tag="big", name=f"ps1_{g}_{mc}") for mc in range(2)]
                for mc in range(2):
                    for dh in range(3):
                        nc.tensor.matmul(
                            ps1[mc][:, :],
                            w1s[:, dh, mc, :],
                            X3r[:, r0 + dh:r0 + dh + ROWS, 1:129],
                            start=(dh == 0),
                            stop=(dh == 2),
                        )
                ds = []
                for mc in range(2):
                    d = acts.tile([128, ROWS * W], MM_DT, tag="act", name=f"ds_{g}_{mc}")
                    nc.scalar.activation(d[:, :], ps1[mc][:, :], RELU,
                                         bias=bbs[:, 0, mc:mc + 1])
                    ds.append(d)
                st[g] = ds

            def stage_mid(g, l):
                ds_prev = st[g]
                ps = [psumb.tile([128, ROWS * W], f32, tag="big", name=f"ps_{g}_{l}_{mc}") for mc in range(2)]
                for mc in range(2):
                    for kc in range(2):
                        nc.tensor.matmul(
                            ps[mc][:, :],
                            wis[:, l, kc, mc, :],
                            ds_prev[kc][:, :],
                            start=(kc == 0),
                            stop=(kc == 1),
                        )
                ds_new = []
                for mc in range(2):
                    d = acts.tile([128, ROWS * W], MM_DT, tag="act", name=f"dsn_{g}_{l}_{mc}")
                    if l < 2 and mc == 0:
                        nc.vector.tensor_scalar(
                            d[:, :], ps[mc][:, :], bbs[:, l + 1, mc:mc + 1],
                            0.0, mybir.AluOpType.add, mybir.AluOpType.max)
                    else:
                        nc.scalar.activation(d[:, :], ps[mc][:, :], RELU,
                                             bias=bbs[:, l + 1, mc:mc + 1])
                    ds_new.append(d)
                st[g] = ds_new

            def stage4(g):
                s, k = divmod(g, NCH)
                ds_prev = st.pop(g)
                ps5 = psum5p.tile([96, ROWS, W], f32, tag="small", name=f"ps5_{g}")
                for kc in range(2):
                    nc.tensor.matmul(
                        ps5[:, :, :],
                        wls[:, kc, :],
                        ds_prev[kc][:, :],
                        start=(kc == 0),
                        stop=(kc == 1),
                    )
                emit_update(k, ps5)
                emit_hist(k, s)

            for slot in range(total + 4):
                if slot < total:
                    stage0(slot)
                if 1 <= slot < total + 1:
                    stage_mid(slot - 1, 0)
                if 2 <= slot < total + 2:
                    stage_mid(slot - 2, 1)
                if 3 <= slot < total + 3:
                    stage_mid(slot - 3, 2)
                if slot >= 4:
                    stage4(slot - 4)

    if not nc.is_finalized():
        nc.finalize()
    return nc


def prepare_host_inputs(x, conv_w, w_first, b_first, w_interim, b_interim, w_last):
    """Pack weights into the kernel's SBUF layouts (numpy, fp32)."""
    conv_w = np.asarray(conv_w, np.float32)
    w_first = np.asarray(w_first, np.float32)
    b_first = np.asarray(b_first, np.float32)
    w_interim = np.asarray(w_interim, np.float32)
    b_interim = np.asarray(b_interim, np.float32)
    w_last = np.asarray(w_last, np.float32)

    # W1c[dh, 16*dw + c, m] = sum_o conv_w[o, c, kh=dw, kw=dh] * w_first[m, o]
    t = np.einsum("ocuv,mo->vucm", conv_w, w_first)      # [dh, dw, c, m]
    W1c = np.zeros((3, 96, HID), np.float32)
    for dw in range(3):
        W1c[:, 32 * dw:32 * dw + 16, :] = t[:, dw]
    w1_host = np.ascontiguousarray(
        W1c.transpose(1, 0, 2).reshape(96, 3, 2, 128))    # [k, dh, mc, mm]

    wi_host = np.ascontiguousarray(
        w_interim.reshape(NL, 2, 128, 2, 128).transpose(4, 0, 3, 1, 2))
    # wi_host[kk, l, kc, mc, mm] = w_interim[l, mc*128+mm, kc*128+kk]

    wl_t = w_last.reshape(16, 2, 128).transpose(2, 1, 0)   # [k, kc, c]
    wl_host = np.zeros((128, 2, 96), np.float32)
    for t_ in range(3):
        wl_host[:, :, 32 * t_:32 * t_ + 16] = wl_t

    b_all = [b_first] + [b_interim[l] for l in range(NL)]
    bias_host = np.ascontiguousarray(
        np.stack([b.reshape(2, 128).T for b in b_all], axis=1))  # [128, li, mc]

    x = np.asarray(x, np.float32)
    wdt = mybir.dt.np(MM_DT)
    xb_cores = []
    xbr_cores = []
    for i in range(B):
        hp = np.zeros((C, Hp, Wp), np.float32)
        hp[:, 1:129, 1:129] = x[i].transpose(2, 0, 1)
        xb = np.zeros((3, 32, Hp, Wp), np.float32)
        xb[0, :C, :, 1:130] = hp[:, :, 0:129]
        xb[1, :C] = hp
        xb[2, :C, :, 0:129] = hp[:, :, 1:130]
        xb_cores.append(xb)
        xbr_cores.append(xb.astype(wdt))
    w1_host = w1_host.astype(wdt)
    wi_host = wi_host.astype(wdt)
    wl_host = wl_host.astype(wdt)
    return xb_cores, xbr_cores, w1_host, wi_host, wl_host, bias_host


def run(inputs: dict, trace: bool = False):
    """Build, run on 8 cores, return ((x_final, history), BassKernelResults)."""
    x = np.asarray(inputs["x"], np.float32)
    steps = int(inputs["steps"])
    if steps == 0:
        hist = x[None].copy()
        return (x.copy(), hist), None

    xb_cores, xbr_cores, w1h, wih, wlh, bh = prepare_host_inputs(
        x, inputs["conv_w"], inputs["w_first"], inputs["b_first"],
        inputs["w_interim"], inputs["b_interim"], inputs["w_last"])

    nc = build_program(steps)
    in_maps = [
        {"xb": xb_cores[i], "xbr": xbr_cores[i], "w1": w1h, "wi": wih,
         "wl": wlh, "bias": bh}
        for i in range(NCORES)
    ]
    res = run_bass_kernel_spmd(nc, in_maps, list(range(NCORES)), trace=trace)

    history = np.empty((steps + 1, B, H, W, C), np.float32)
    history[0] = x
    for i in range(NCORES):
        hist_i = res.results[i]["hist"]  # [steps, C, H, W]
        history[1:, i] = hist_i.transpose(0, 2, 3, 1)
    x_final = history[steps].copy()
    return (x_final, history), res


def kernel(x, conv_w, w_first, b_first, w_interim, b_interim, w_last, steps):
    (x_final, history), _ = run(dict(
        x=x, conv_w=conv_w, w_first=w_first, b_first=b_first,
        w_interim=w_interim, b_interim=b_interim, w_last=w_last, steps=steps))
    return x_final, history
